# revision 14
# baseline (speedup 1.0000x reference)
"""LIF layer (T=64, B=128, 2048->2048) on 8 trn2 NeuronCores.

Sharding: 4-way over out_dim x 2-way over batch. Core (g, h) owns
out channels [g*512, (g+1)*512) and batch rows [h*64, (h+1)*64).

Per core:
  GEMM  cur[o, (t,b)] = sum_i W[o,i] * x[t,b,i] as a single f32r pass
        (fp22-ish precision, 1 cyc/row, measured ~0.015 rel on spikes)
  SCAN  64 sequential LIF steps on [128, 4, 64] state tiles (DVE),
        reading cur straight out of PSUM.
Bias is folded away via the change of variable u = mem - b/(1-decay),
turning the per-step bias add into a per-channel spike threshold
(THR=1 so the reset subtract is just u -= spk).

Col-blocks sized [512 x7, 384, 128] (8/8/.../6/2 timesteps): PSUM
tile [128, 4ot, 512] = 4 banks, double-buffered. Steady-state pair
rate is ~227 ns per 512-col f32r matmul (the 518-cycle back-to-back
roofline); LDWEIGHTS hides behind the previous matmul's column
stream. The taper trades ~4us of extra LDWEIGHTS overhead in the
last two blocks for cutting the serial post-GEMM tail from 8 scan
steps (~11us on DVE) to 2.

Prologue discipline (HBM is ~425 GB/s aggregate across queues, SWDGE
descriptor gen is ~5 ns per partition-line): the junk warmup tile is
memset on-chip (no DMA) so the PE HAM ramp starts right after the
framework preamble; W chunk 0 goes first on the gpsimd ring (u0/thr
after it); the remaining W chunks interleave with block-0 x on the
sync ring in exact consumption order. Blocks 0-2 run kt-outer so
matmuls start as soon as each (W, x) chunk pair lands (x for blocks
1-2 is still in flight when they start); junk matmuls bridge the
supply-limited stretches so the PE's HAM clock stays at 2.4 GHz.
Junk matmuls multiply zeros and accumulate (start=False) into the
current block's PSUM tile, so they are numeric no-ops wherever they
land. Blocks 3+ are PE-bound and run ot-outer.

Host-side prep: x is packed flat per block ([128, sum_i KT*blk_i]
with each (block, kt-chunk) slice one contiguous line per partition,
so every chunk DMA is cheap to descriptor-gen), W sliced/packed,
threshold/init tiles precomputed; output spikes return as bf16 (on
the idle Scalar engine's DMA ring) and are reassembled/cast on the
host.
"""

import math

import numpy as np

import concourse.bacc as bacc
import concourse.mybir as mybir
import concourse.tile as tile
from concourse import bass_utils

# Problem constants (hardcoded per contract)
T, B, I, O = 64, 128, 2048, 2048
N_CORES = 8
GO, GB = 4, 2              # out-groups x batch-groups
OL = O // GO               # 512 out-channels per core
OT = OL // 128             # 4 out tiles
BL = B // GB               # 64 batch rows per core
COLS = T * BL              # 4096 (t,b) columns per core
KT = I // 128              # 16 k-tiles
BLKS = [512] * 7 + [448, 64]           # tapered col-blocks (sum = COLS)
OFFS = [sum(BLKS[:i]) for i in range(len(BLKS))]
BLKMAX = max(BLKS)
TAU, THR = 2.0, 1.0
DECAY = math.exp(-1.0 / TAU)

F32 = mybir.dt.float32
F32R = mybir.dt.float32r
BF16 = mybir.dt.bfloat16
ALU = mybir.AluOpType

MODE = "f32r-tp4dp2-v9"

N_WARMUP = 12              # junk matmuls burning the HAM ramp
CHUNKED_BLOCKS = 3         # blocks 0..2 run kt-outer, supply-chunked
# Staircased kt-chunk bounds per chunked block: block 0 alternates 2-kt
# x/W chunks in exact consumption order (the supply window is the one
# stretch where every lump directly stalls the PE); later blocks coarsen.
# Size-classed pool tags keep ring slots small so blocks 3+ DMAs issue
# early (ring slots gate descriptor generation on the sync engine).
X_BOUNDS = {0: [0, 2, 4, 6, 8, 10, 12, 14, 16], 1: [0, 4, 8, 12, 16]}
X_TAG = {0: ("xts", 2), 1: ("xtm", 4)}        # tag, kt-slot-size per block
X_TAG_DEFAULT = ("xtl", 8)
BRIDGE_AT_START = {1: 8, 2: 4, 3: 4}  # junk count at block boundaries

_cache = {}


def _build_nc():
    nc = bacc.Bacc(trn_type="TRN2", target_bir_lowering=False)

    xT_d = nc.dram_tensor("xT", [128, KT * COLS], F32R, kind="ExternalInput")
    w_d = nc.dram_tensor("w", [128, KT, OT, 128], F32R, kind="ExternalInput")
    thr_d = nc.dram_tensor("thr", [128, OT, BL], F32, kind="ExternalInput")
    u0_d = nc.dram_tensor("u0", [128, OT, BL], F32, kind="ExternalInput")
    out_d = nc.dram_tensor("out", [128, T, OT, BL], BF16, kind="ExternalOutput")

    with tile.TileContext(nc) as tc:
        with (
            tc.tile_pool(name="wpool", bufs=1) as wpool,
            tc.tile_pool(name="xpool_s", bufs=8) as xpool_s,
            tc.tile_pool(name="xpool_m", bufs=5) as xpool_m,
            tc.tile_pool(name="xpool_l", bufs=5) as xpool_l,
            tc.tile_pool(name="state", bufs=1) as state,
            tc.tile_pool(name="spkpool", bufs=2) as spkpool,
            tc.tile_pool(name="psum", bufs=2, space="PSUM") as psum_pool,
        ):
            # Junk tile is memset on-chip: the warmup matmuls that ramp the
            # PE HAM clock start right after the framework preamble instead
            # of waiting ~3us for a DMA round-trip.
            junk_f = state.tile([128, 128], F32)
            nc.gpsimd.memset(junk_f[:], 0.0)
            junk = junk_f[:].bitcast(F32R)

            # W chunk 0 rides the near-empty gpsimd ring FIRST (it gates
            # the first real matmul); u0/thr follow on the same ring (they
            # are not needed until the first scan at ~25us).
            w_chunks = []
            wc0 = wpool.tile([128, 2, OT, 128], F32R, name="w_0")
            nc.gpsimd.dma_start(wc0[:], w_d[:, 0:2])
            w_chunks.append((0, wc0))

            u = state.tile([128, OT, BL], F32)
            thr_t = state.tile([128, OT, BL], F32)
            nc.gpsimd.dma_start(u[:], u0_d[:])
            nc.gpsimd.dma_start(thr_t[:], thr_d[:])

            def w_tile(kt):
                for lo, wc in reversed(w_chunks):
                    if kt >= lo:
                        return wc[:, kt - lo]
                raise AssertionError

            def bridge(ps, n):
                # Zeros x zeros accumulated with start=False: a numeric
                # no-op that keeps the PE HAM activity window warm.
                for _ in range(n):
                    nc.tensor.matmul(ps[:, 0, :128], junk, junk,
                                     start=False, stop=False)

            nblk = len(BLKS)
            for bi in range(nblk):
                blk = BLKS[bi]
                tblk = blk // BL
                toff = OFFS[bi] // BL
                # x for this block: kt-chunked DMAs out of the flat pack.
                # Chunked blocks are staircased (and, for block 0,
                # interleaved with the W chunks in exact consumption order
                # on the sync ring).
                x_bounds = X_BOUNDS.get(bi, [0, 8, KT])
                tag, slot_kt = X_TAG.get(bi, X_TAG_DEFAULT)
                pool = {"xts": xpool_s, "xtm": xpool_m,
                        "xtl": xpool_l}[tag]
                xts = []
                for xi in range(len(x_bounds) - 1):
                    lo, hi = x_bounds[xi], x_bounds[xi + 1]
                    if bi == 0 and xi > 0:
                        wc = wpool.tile([128, hi - lo, OT, 128], F32R,
                                        name=f"w_{xi}")
                        nc.sync.dma_start(wc[:], w_d[:, lo:hi])
                        w_chunks.append((lo, wc))
                    xt = pool.tile([128, slot_kt * BLKMAX], F32R, tag=tag,
                                   name=f"xt_{bi}_{xi}")
                    nc.sync.dma_start(
                        xt[:, :(hi - lo) * blk],
                        xT_d[:, KT * OFFS[bi] + lo * blk:
                                KT * OFFS[bi] + hi * blk])
                    xts.append((lo, hi, xt))

                def x_slice(kt):
                    for lo, hi, xt in xts:
                        if lo <= kt < hi:
                            return xt[:, (kt - lo) * blk:(kt - lo + 1) * blk]
                    raise AssertionError

                ps = psum_pool.tile([128, OT, BLKMAX], F32, tag="ps",
                                    name=f"ps_{bi}")
                if bi == 0:
                    bridge(ps, N_WARMUP)
                elif bi in BRIDGE_AT_START:
                    bridge(ps, BRIDGE_AT_START[bi])

                # Chunked blocks run kt-outer so matmuls start as soon as
                # each (W, x) chunk pair lands; later blocks run ot-outer
                # for long same-bank accumulation runs.
                if bi < CHUNKED_BLOCKS:
                    order = [(ot, kt) for kt in range(KT) for ot in range(OT)]
                else:
                    order = [(ot, kt) for ot in range(OT) for kt in range(KT)]
                junk_after = ({1: 5, 3: 4, 5: 3, 7: 3, 9: 2, 11: 2, 13: 1}
                              if bi == 0 else {})
                for ot, kt in order:
                    nc.tensor.matmul(
                        ps[:, ot, :blk],
                        w_tile(kt)[:, ot, :],
                        x_slice(kt),
                        start=(kt == 0),
                        stop=(kt == KT - 1),
                    )
                    # Bridge supply-limited chunk boundaries with junk
                    # matmuls so the HAM activity window stays warm.
                    if ot == OT - 1 and kt in junk_after:
                        bridge(ps, junk_after[kt])

                # LIF steps consuming this block's PSUM; spikes land in
                # half-block buffers so the out-DMA (idle Scalar engine's
                # ring) overlaps the scan. The last (tapered, 2-step) block
                # uses per-step buffers/DMAs so only one step's transfer
                # trails the final scan op, and skips the dead final
                # membrane update.
                last = bi == nblk - 1
                spkb = spkpool.tile([128, tblk, OT, BL], BF16,
                                    tag=f"spk{tblk}", name=f"spk_{bi}")
                for tl in range(tblk):
                    nc.vector.scalar_tensor_tensor(
                        u[:], u[:], DECAY,
                        ps[:, :, tl * BL:(tl + 1) * BL],
                        op0=ALU.mult, op1=ALU.add)
                    nc.vector.tensor_tensor(
                        spkb[:, tl], u[:], thr_t[:], op=ALU.is_gt)
                    if not (last and tl == tblk - 1):
                        nc.vector.tensor_tensor(
                            u[:], u[:], spkb[:, tl], op=ALU.subtract)
                nc.scalar.dma_start(out_d[:, toff:toff + tblk], spkb[:])

    nc.compile()
    return nc


def _get_nc():
    if "nc" not in _cache:
        _cache["nc"] = _build_nc()
    return _cache["nc"]


def kernel(x_seq: np.ndarray, W: np.ndarray, b: np.ndarray) -> np.ndarray:
    nc = _get_nc()

    # Two distinct x shards (one per batch half), shared by 4 cores each.
    # Packed flat [128(p), sum_i KT*blk_i]: per block a [KT, blk] region,
    # so each (block, kt-chunk) DMA is one contiguous line per partition.
    xTs = []
    for h in range(GB):
        xs = np.ascontiguousarray(
            x_seq[:, h * BL:(h + 1) * BL, :], dtype=np.float32)
        xT = xs.reshape(T * BL, I).T.reshape(KT, 128, COLS)  # [KT,128,COLS]
        parts = []
        for bi, blk in enumerate(BLKS):
            blkv = xT[:, :, OFFS[bi]:OFFS[bi] + blk]         # [KT,128,blk]
            parts.append(blkv.transpose(1, 0, 2).reshape(128, KT * blk))
        xTs.append(np.ascontiguousarray(np.concatenate(parts, axis=1)))

    in_maps = []
    for c in range(N_CORES):
        g, h = c // GB, c % GB
        w_c = W[g * OL:(g + 1) * OL, :].astype(np.float32)      # [OL, I]
        wTc = np.ascontiguousarray(w_c.T)                       # [I, OL]
        wp = np.ascontiguousarray(
            wTc.reshape(KT, 128, OT, 128).transpose(1, 0, 2, 3))
        b_c = b[g * OL:(g + 1) * OL].astype(np.float32)         # [OL]
        shift = b_c / (1.0 - DECAY)
        thr = (THR - shift).reshape(OT, 128).T                  # [128, OT]
        u0 = (-shift).reshape(OT, 128).T
        thr_tile = np.ascontiguousarray(
            np.broadcast_to(thr[:, :, None], (128, OT, BL)), dtype=np.float32)
        u0_tile = np.ascontiguousarray(
            np.broadcast_to(u0[:, :, None], (128, OT, BL)), dtype=np.float32)
        in_maps.append({
            "xT": xTs[h], "w": wp, "thr": thr_tile, "u0": u0_tile,
        })

    res = bass_utils.run_bass_kernel_spmd(nc, in_maps, core_ids=list(range(N_CORES)))
    global LAST_RESULT
    LAST_RESULT = res

    # Assemble: out_c[op, t, ot, b] -> [t, b, ot*128+op] per core block
    out = np.empty((T, B, O), dtype=np.float32)
    for c in range(N_CORES):
        g, h = c // GB, c % GB
        oc = res.results[c]["out"].astype(np.float32)  # [128, T, OT, BL]
        out[:, h * BL:(h + 1) * BL, g * OL:(g + 1) * OL] = (
            oc.transpose(1, 3, 2, 0).reshape(T, BL, OL))
    return out


LAST_RESULT = None


# revision 15
# speedup vs baseline: 1.0223x; 1.0223x over previous
"""LIF layer (T=64, B=128, 2048->2048) on 8 trn2 NeuronCores.

Sharding: 4-way over out_dim x 2-way over batch. Core (g, h) owns
out channels [g*512, (g+1)*512) and batch rows [h*64, (h+1)*64).

Per core:
  GEMM  cur[o, (t,b)] = sum_i W[o,i] * x[t,b,i] as a single f32r pass
        (fp22-ish precision, 1 cyc/row, measured ~0.015 rel on spikes)
  SCAN  64 sequential LIF steps on [128, 4, 64] state tiles (DVE),
        reading cur straight out of PSUM.
Bias is folded away via the change of variable u = mem - b/(1-decay),
turning the per-step bias add into a per-channel spike threshold
(THR=1 so the reset subtract is just u -= spk).

Col-blocks sized [512 x7, 384, 128] (8/8/.../6/2 timesteps): PSUM
tile [128, 4ot, 512] = 4 banks, double-buffered. Steady-state pair
rate is ~227 ns per 512-col f32r matmul (the 518-cycle back-to-back
roofline); LDWEIGHTS hides behind the previous matmul's column
stream. The taper trades ~4us of extra LDWEIGHTS overhead in the
last two blocks for cutting the serial post-GEMM tail from 8 scan
steps (~11us on DVE) to 2.

Prologue discipline (HBM is ~425 GB/s aggregate across queues, SWDGE
descriptor gen is ~5 ns per partition-line): the junk warmup tile is
memset on-chip (no DMA) so the PE HAM ramp starts right after the
framework preamble; W chunk 0 goes first on the gpsimd ring (u0/thr
after it); the remaining W chunks interleave with block-0 x on the
sync ring in exact consumption order. Blocks 0-2 run kt-outer so
matmuls start as soon as each (W, x) chunk pair lands (x for blocks
1-2 is still in flight when they start); junk matmuls bridge the
supply-limited stretches so the PE's HAM clock stays at 2.4 GHz.
Junk matmuls multiply zeros and accumulate (start=False) into the
current block's PSUM tile, so they are numeric no-ops wherever they
land. Blocks 3+ are PE-bound and run ot-outer.

Host-side prep: x is packed flat per block ([128, sum_i KT*blk_i]
with each (block, kt-chunk) slice one contiguous line per partition,
so every chunk DMA is cheap to descriptor-gen), W sliced/packed,
threshold/init tiles precomputed; output spikes return as bf16 (on
the idle Scalar engine's DMA ring) and are reassembled/cast on the
host.
"""

import math

import numpy as np

import concourse.bacc as bacc
import concourse.mybir as mybir
import concourse.tile as tile
from concourse import bass_utils

# Problem constants (hardcoded per contract)
T, B, I, O = 64, 128, 2048, 2048
N_CORES = 8
GO, GB = 4, 2              # out-groups x batch-groups
OL = O // GO               # 512 out-channels per core
OT = OL // 128             # 4 out tiles
BL = B // GB               # 64 batch rows per core
COLS = T * BL              # 4096 (t,b) columns per core
KT = I // 128              # 16 k-tiles
BLKS = [512] * 7 + [384, 128]          # tapered col-blocks (sum = COLS)
OFFS = [sum(BLKS[:i]) for i in range(len(BLKS))]
BLKMAX = max(BLKS)
TAU, THR = 2.0, 1.0
DECAY = math.exp(-1.0 / TAU)

F32 = mybir.dt.float32
F32R = mybir.dt.float32r
BF16 = mybir.dt.bfloat16
ALU = mybir.AluOpType

MODE = "f32r-tp4dp2-v10"

N_WARMUP = 12              # junk matmuls burning the HAM ramp
CHUNKED_BLOCKS = 3         # blocks 0..2 run kt-outer, supply-chunked
# Staircased kt-chunk bounds per chunked block: block 0 alternates 2-kt
# x/W chunks in exact consumption order (the supply window is the one
# stretch where every lump directly stalls the PE); later blocks coarsen.
# Size-classed pool tags keep ring slots small so blocks 3+ DMAs issue
# early (ring slots gate descriptor generation on the sync engine).
X_BOUNDS = {0: [0, 2, 4, 6, 8, 10, 12, 14, 16], 1: [0, 4, 8, 12, 16]}
X_TAG = {0: ("xts", 2), 1: ("xtm", 4)}        # tag, kt-slot-size per block
X_TAG_DEFAULT = ("xtl", 8)
BRIDGE_AT_START = {1: 8, 2: 4, 3: 4}  # junk count at block boundaries

_cache = {}


def _build_nc():
    nc = bacc.Bacc(trn_type="TRN2", target_bir_lowering=False)

    xT_d = nc.dram_tensor("xT", [128, KT * COLS], F32R, kind="ExternalInput")
    w_d = nc.dram_tensor("w", [128, KT, OT, 128], F32R, kind="ExternalInput")
    thr_d = nc.dram_tensor("thr", [128, OT, BL], F32, kind="ExternalInput")
    u0_d = nc.dram_tensor("u0", [128, OT, BL], F32, kind="ExternalInput")
    out_d = nc.dram_tensor("out", [128, T, OT, BL], BF16, kind="ExternalOutput")

    with tile.TileContext(nc) as tc:
        with (
            tc.tile_pool(name="wpool", bufs=1) as wpool,
            tc.tile_pool(name="xpool_s", bufs=8) as xpool_s,
            tc.tile_pool(name="xpool_m", bufs=5) as xpool_m,
            tc.tile_pool(name="xpool_l", bufs=5) as xpool_l,
            tc.tile_pool(name="state", bufs=1) as state,
            tc.tile_pool(name="spkpool", bufs=4) as spkpool,
            tc.tile_pool(name="psum", bufs=2, space="PSUM") as psum_pool,
        ):
            # Junk tile is memset on-chip: the warmup matmuls that ramp the
            # PE HAM clock start right after the framework preamble instead
            # of waiting ~3us for a DMA round-trip.
            junk_f = state.tile([128, 128], F32)
            nc.gpsimd.memset(junk_f[:], 0.0)
            junk = junk_f[:].bitcast(F32R)

            # W chunk 0 rides the near-empty gpsimd ring FIRST (it gates
            # the first real matmul); u0/thr follow on the same ring (they
            # are not needed until the first scan at ~25us).
            w_chunks = []
            wc0 = wpool.tile([128, 2, OT, 128], F32R, name="w_0")
            nc.gpsimd.dma_start(wc0[:], w_d[:, 0:2])
            w_chunks.append((0, wc0))

            u = state.tile([128, OT, BL], F32)
            thr_t = state.tile([128, OT, BL], F32)
            nc.gpsimd.dma_start(u[:], u0_d[:])
            nc.gpsimd.dma_start(thr_t[:], thr_d[:])

            def w_tile(kt):
                for lo, wc in reversed(w_chunks):
                    if kt >= lo:
                        return wc[:, kt - lo]
                raise AssertionError

            def bridge(ps, n):
                # Zeros x zeros accumulated with start=False: a numeric
                # no-op that keeps the PE HAM activity window warm.
                for _ in range(n):
                    nc.tensor.matmul(ps[:, 0, :128], junk, junk,
                                     start=False, stop=False)

            nblk = len(BLKS)
            for bi in range(nblk):
                blk = BLKS[bi]
                tblk = blk // BL
                toff = OFFS[bi] // BL
                # x for this block: kt-chunked DMAs out of the flat pack.
                # Chunked blocks are staircased (and, for block 0,
                # interleaved with the W chunks in exact consumption order
                # on the sync ring).
                x_bounds = X_BOUNDS.get(bi, [0, 8, KT])
                tag, slot_kt = X_TAG.get(bi, X_TAG_DEFAULT)
                pool = {"xts": xpool_s, "xtm": xpool_m,
                        "xtl": xpool_l}[tag]
                xts = []
                for xi in range(len(x_bounds) - 1):
                    lo, hi = x_bounds[xi], x_bounds[xi + 1]
                    if bi == 0 and xi > 0:
                        wc = wpool.tile([128, hi - lo, OT, 128], F32R,
                                        name=f"w_{xi}")
                        nc.sync.dma_start(wc[:], w_d[:, lo:hi])
                        w_chunks.append((lo, wc))
                    xt = pool.tile([128, slot_kt * BLKMAX], F32R, tag=tag,
                                   name=f"xt_{bi}_{xi}")
                    nc.sync.dma_start(
                        xt[:, :(hi - lo) * blk],
                        xT_d[:, KT * OFFS[bi] + lo * blk:
                                KT * OFFS[bi] + hi * blk])
                    xts.append((lo, hi, xt))

                def x_slice(kt):
                    for lo, hi, xt in xts:
                        if lo <= kt < hi:
                            return xt[:, (kt - lo) * blk:(kt - lo + 1) * blk]
                    raise AssertionError

                ps = psum_pool.tile([128, OT, BLKMAX], F32, tag="ps",
                                    name=f"ps_{bi}")
                if bi == 0:
                    bridge(ps, N_WARMUP)
                elif bi in BRIDGE_AT_START:
                    bridge(ps, BRIDGE_AT_START[bi])

                # Chunked blocks run kt-outer so matmuls start as soon as
                # each (W, x) chunk pair lands; later blocks run ot-outer
                # for long same-bank accumulation runs.
                if bi < CHUNKED_BLOCKS:
                    order = [(ot, kt) for kt in range(KT) for ot in range(OT)]
                else:
                    order = [(ot, kt) for ot in range(OT) for kt in range(KT)]
                junk_after = ({1: 6, 3: 5, 5: 4, 7: 4, 9: 3, 11: 2, 13: 2}
                              if bi == 0 else {})
                for ot, kt in order:
                    nc.tensor.matmul(
                        ps[:, ot, :blk],
                        w_tile(kt)[:, ot, :],
                        x_slice(kt),
                        start=(kt == 0),
                        stop=(kt == KT - 1),
                    )
                    # Bridge supply-limited chunk boundaries with junk
                    # matmuls so the HAM activity window stays warm.
                    if ot == OT - 1 and kt in junk_after:
                        bridge(ps, junk_after[kt])

                # LIF steps consuming this block's PSUM; spikes land in
                # half-block buffers so the out-DMA (idle Scalar engine's
                # ring) overlaps the scan. The last (tapered, 2-step) block
                # uses per-step buffers/DMAs so only one step's transfer
                # trails the final scan op, and skips the dead final
                # membrane update.
                last = bi == nblk - 1
                steps_per_buf = 1 if tblk <= 2 else tblk // 2
                out_eng = nc.sync if bi >= nblk - 2 else nc.scalar
                for hf in range(tblk // steps_per_buf):
                    spkb = spkpool.tile([128, steps_per_buf, OT, BL], BF16,
                                        tag="spk1" if last else "spk",
                                        name=f"spk_{bi}_{hf}")
                    for tj in range(steps_per_buf):
                        tl = hf * steps_per_buf + tj
                        nc.vector.scalar_tensor_tensor(
                            u[:], u[:], DECAY,
                            ps[:, :, tl * BL:(tl + 1) * BL],
                            op0=ALU.mult, op1=ALU.add)
                        nc.vector.tensor_tensor(
                            spkb[:, tj], u[:], thr_t[:], op=ALU.is_gt)
                        if not (last and tl == tblk - 1):
                            nc.vector.tensor_tensor(
                                u[:], u[:], spkb[:, tj], op=ALU.subtract)
                    t0 = toff + hf * steps_per_buf
                    out_eng.dma_start(
                        out_d[:, t0:t0 + steps_per_buf], spkb[:])

    nc.compile()
    return nc


def _get_nc():
    if "nc" not in _cache:
        _cache["nc"] = _build_nc()
    return _cache["nc"]


def kernel(x_seq: np.ndarray, W: np.ndarray, b: np.ndarray) -> np.ndarray:
    nc = _get_nc()

    # Two distinct x shards (one per batch half), shared by 4 cores each.
    # Packed flat [128(p), sum_i KT*blk_i]: per block a [KT, blk] region,
    # so each (block, kt-chunk) DMA is one contiguous line per partition.
    xTs = []
    for h in range(GB):
        xs = np.ascontiguousarray(
            x_seq[:, h * BL:(h + 1) * BL, :], dtype=np.float32)
        xT = xs.reshape(T * BL, I).T.reshape(KT, 128, COLS)  # [KT,128,COLS]
        parts = []
        for bi, blk in enumerate(BLKS):
            blkv = xT[:, :, OFFS[bi]:OFFS[bi] + blk]         # [KT,128,blk]
            parts.append(blkv.transpose(1, 0, 2).reshape(128, KT * blk))
        xTs.append(np.ascontiguousarray(np.concatenate(parts, axis=1)))

    in_maps = []
    for c in range(N_CORES):
        g, h = c // GB, c % GB
        w_c = W[g * OL:(g + 1) * OL, :].astype(np.float32)      # [OL, I]
        wTc = np.ascontiguousarray(w_c.T)                       # [I, OL]
        wp = np.ascontiguousarray(
            wTc.reshape(KT, 128, OT, 128).transpose(1, 0, 2, 3))
        b_c = b[g * OL:(g + 1) * OL].astype(np.float32)         # [OL]
        shift = b_c / (1.0 - DECAY)
        thr = (THR - shift).reshape(OT, 128).T                  # [128, OT]
        u0 = (-shift).reshape(OT, 128).T
        thr_tile = np.ascontiguousarray(
            np.broadcast_to(thr[:, :, None], (128, OT, BL)), dtype=np.float32)
        u0_tile = np.ascontiguousarray(
            np.broadcast_to(u0[:, :, None], (128, OT, BL)), dtype=np.float32)
        in_maps.append({
            "xT": xTs[h], "w": wp, "thr": thr_tile, "u0": u0_tile,
        })

    res = bass_utils.run_bass_kernel_spmd(nc, in_maps, core_ids=list(range(N_CORES)))
    global LAST_RESULT
    LAST_RESULT = res

    # Assemble: out_c[op, t, ot, b] -> [t, b, ot*128+op] per core block
    out = np.empty((T, B, O), dtype=np.float32)
    for c in range(N_CORES):
        g, h = c // GB, c % GB
        oc = res.results[c]["out"].astype(np.float32)  # [128, T, OT, BL]
        out[:, h * BL:(h + 1) * BL, g * OL:(g + 1) * OL] = (
            oc.transpose(1, 3, 2, 0).reshape(T, BL, OL))
    return out


LAST_RESULT = None
